# revision 11
# baseline (speedup 1.0000x reference)
"""Causal self-attention (B=4, S=2048, D=768, H=12) on 8 TRN2 NeuronCores.

Sharding: core = (batch b in 0..3) x (head-group hg in 0..1, 6 heads each).
Host pre-transposes x -> xT per batch, slices w_qkv columns / w_proj rows per
head-group.  Each core computes its 6 heads end-to-end and a partial
projection output [S, D]; the host sums the two head-group partials per batch
and adds b_proj.

Device layouts (per core):
  xT   [768, 2048]   (d on partitions)  -> 6 sbuf tiles [128, S]
  qkT  [768(qk cols), S]: rows 0-383 = qT (6 heads x 64), 384-767 = kT.
       6 tiles [128, S]; tile hp (0-2) = qT of head pair hp, tile 3+hp = kT.
  v    natural [S, 6, 65]: per s-tile [128, 6, 65]; col 64 of each head block
       is 1.0 -> the attn @ [v|1] matmul emits the softmax denominator row.
  scores computed TRANSPOSED: sT[kpos, qpos] = k . q  (lhsT=kT, rhs=qT,
       row-tiled pair: head0 at partitions 0-63, head1 at 64-127 run
       concurrently in the PE array).  Softmax denom = row 64 of yT psum.
  yT   [128 (pair y-dims), S] per pair -> proj lhsT directly.
"""

import numpy as np
from contextlib import ExitStack

import concourse.bass as bass
import concourse.bacc as bacc
import concourse.mybir as mybir
from concourse.tile import TileContext

F32 = mybir.dt.float32
F32R = mybir.dt.float32r

D = 768
NCORES = 8
SCALE = 0.125  # 1/sqrt(64)


def build_program(S=2048, use_f32r=True):
    NS = S // 512   # q strips
    NT = S // 128   # s tiles
    DT = D // 128   # d tiles (contraction)

    nc = bacc.Bacc()

    MDT = F32R if use_f32r else F32  # matmul input dtype

    xT = nc.dram_tensor("xT_s", [D, S], MDT, kind="ExternalInput")
    wqkv = nc.dram_tensor("wqkv_s", [D, 1152], MDT, kind="ExternalInput")
    bqk = nc.dram_tensor("bqk_s", [128, 6], F32, kind="ExternalInput")
    bv = nc.dram_tensor("bv_s", [1, 384], MDT, kind="ExternalInput")
    wproj = nc.dram_tensor("wproj_s", [384, D], MDT, kind="ExternalInput")
    out = nc.dram_tensor("out_s", [S, D], F32, kind="ExternalOutput")

    def r(ap):
        return ap

    with TileContext(nc) as tc, ExitStack() as ctx:
        persist = ctx.enter_context(tc.tile_pool(name="persist", bufs=1))

        qkT = [persist.tile([128, S], MDT, tag=f"qkT{i}", name=f"qkT{i}")
               for i in range(6)]
        v_sb = [persist.tile([128, 6, 65], MDT, tag=f"v{i}", name=f"v{i}")
                for i in range(NT)]
        yT = [persist.tile([128, S], MDT, tag=f"yT{i}", name=f"yT{i}")
              for i in range(3)]
        wp = [persist.tile([128, D], MDT, tag=f"wp{i}", name=f"wp{i}")
              for i in range(3)]
        bqk_sb = persist.tile([128, 6], F32, tag="bqk", name="bqk_sb")
        bv_sb = persist.tile([1, 384], MDT, tag="bv", name="bv_sb")
        ones = persist.tile([1, 128], MDT, tag="ones", name="ones_sb")
        ones_f = persist.tile([1, 64], F32, tag="ones_f", name="ones_f_sb")

        nc.vector.memset(ones[:].bitcast(F32), 1.0)
        nc.vector.memset(ones_f[:], 1.0)
        nc.sync.dma_start(out=bqk_sb[:], in_=bqk[:])
        nc.sync.dma_start(out=bv_sb[:], in_=bv[:])
        for i in range(3):
            nc.sync.dma_start(out=wp[i][:], in_=wproj[128 * i:128 * (i + 1), :])
        for st in range(NT):
            nc.vector.memset(v_sb[st][:, :, 64:65].bitcast(F32), 1.0)

        ps_mm = ctx.enter_context(
            tc.tile_pool(name="ps_mm", bufs=2, space="PSUM"))

        def pe_touch(ap):
            # Tiny self-matmul that makes the PE wait on this tile's producer
            # once, so later real matmuls carry at most ONE sync wait each
            # (the fp32r self-loading matmul has a single LW wait slot).
            t = ps_mm.tile([1, 1], F32, tag="mm", name="touch")
            nc.tensor.matmul(t[:], ap.bitcast(F32), ap.bitcast(F32),
                             start=True, stop=True)

        with tc.tile_pool(name="xw", bufs=1) as xw_pool:
            xT_sb = [xw_pool.tile([128, S], MDT, tag=f"xT{i}", name=f"xTs{i}")
                     for i in range(DT)]
            w_sb = [xw_pool.tile([128, 1152], MDT, tag=f"w{i}", name=f"ws{i}")
                    for i in range(DT)]
            for i in range(DT):
                nc.sync.dma_start(out=xT_sb[i][:],
                                  in_=xT[128 * i:128 * (i + 1), :])
                nc.sync.dma_start(out=w_sb[i][:],
                                  in_=wqkv[128 * i:128 * (i + 1), :])
                pe_touch(xT_sb[i][:, 0:1])
                pe_touch(w_sb[i][:, 0:1])
            for i in range(3):
                pe_touch(wp[i][:, 0:1])

            # ---- Phase 1: qkT[c, s] = sum_d wqkv[d, c] * xT[d, s] + bias ----
            for ns in range(NS):
                for ct in range(6):
                    ps = ps_mm.tile([128, 512], F32, tag="mm", name="ps_qk")
                    for dt_i in range(DT):
                        nc.tensor.matmul(
                            ps[:],
                            r(w_sb[dt_i][:, 128 * ct:128 * ct + 128]),
                            r(xT_sb[dt_i][:, 512 * ns:512 * ns + 512]),
                            start=(dt_i == 0), stop=(dt_i == DT - 1))
                    nc.vector.tensor_scalar_add(
                        qkT[ct][:, 512 * ns:512 * ns + 512], ps[:],
                        bqk_sb[:, ct:ct + 1])

            # ---- Phase 2: v[s, c] = sum_d xT[d, s] * wv[d, c] + bv ----
            for st in range(NT):
                ps = ps_mm.tile([128, 384], F32, tag="mm", name="ps_v")
                for dt_i in range(DT):
                    nc.tensor.matmul(
                        ps[:],
                        r(xT_sb[dt_i][:, 128 * st:128 * st + 128]),
                        r(w_sb[dt_i][:, 768:1152]),
                        start=(dt_i == 0), stop=False)
                nc.tensor.matmul(ps[:], r(ones[:, 0:128]), r(bv_sb[:]),
                                 start=False, stop=True)
                nc.vector.tensor_copy(
                    v_sb[st][:, :, 0:64],
                    ps[:].rearrange("p (h e) -> p h e", h=6))
                pe_touch(v_sb[st][:, 0, 0:1])

        # ---- Phase 3: attention, scores transposed, per head pair ----
        with tc.tile_pool(name="ps_s", bufs=4, space="PSUM") as ps_s, \
             tc.tile_pool(name="ps_y", bufs=2, space="PSUM") as ps_y, \
             tc.tile_pool(name="expp", bufs=6) as expp, \
             tc.tile_pool(name="rcp", bufs=4) as rcp:
            for ns in range(NS):
                q0 = 512 * ns
                for hp in range(3):
                    qt = qkT[hp]
                    kt = qkT[3 + hp]
                    nk = 4 * (ns + 1)
                    yh = [ps_y.tile([65, 512], F32, tag="yh", name="yh0"),
                          ps_y.tile([65, 512], F32, tag="yh", name="yh1")]
                    for kb in range(nk):
                        diag = kb >= 4 * ns
                        c0 = 128 * kb - q0 if diag else 0
                        for h in range(2):
                            p0 = 64 * h
                            sc = ps_s.tile([128, 512], F32, tag="sc",
                                           name="sc")
                            nc.tensor.matmul(
                                sc[:, c0:512],
                                r(kt[p0:p0 + 64, 128 * kb:128 * kb + 128]),
                                r(qt[p0:p0 + 64, q0 + c0:q0 + 512]),
                                start=True, stop=True)
                            ex = expp.tile([128, 512], MDT, tag="exp",
                                           name="ex")
                            nc.scalar.activation(
                                ex[:, c0:512], sc[:, c0:512],
                                mybir.ActivationFunctionType.Exp, scale=SCALE)
                            if diag:
                                # causal triangle -> 0, into a fresh tile so
                                # the yT matmul depends on Pool only
                                ex2 = expp.tile([128, 512], MDT, tag="exp",
                                                name="ex2")
                                nc.gpsimd.affine_select(
                                    out=ex2[:, c0:512],
                                    in_=ex[:, c0:512],
                                    compare_op=mybir.AluOpType.is_ge,
                                    fill=0.0, base=0,
                                    pattern=[[1, 512 - c0]],
                                    channel_multiplier=-1)
                                ex = ex2
                            nc.tensor.matmul(
                                yh[h][:, c0:512],
                                r(v_sb[kb][:, 2 * hp + h, :]),
                                r(ex[:, c0:512]),
                                start=(kb == 0), stop=(kb == nk - 1),
                                skip_group_check=True)
                    for h in range(2):
                        rec = rcp.tile([1, 512], F32, tag="rec", name="rec")
                        nc.vector.reciprocal(rec[:], yh[h][64:65, :])
                        # broadcast 1/l across partitions via ones-matmul
                        # (fp32: multiply by 1.0 is exact)
                        rb = ps_s.tile([64, 512], F32, tag="sc", name="rb")
                        nc.tensor.matmul(rb[:], ones_f[:, 0:64], rec[:],
                                         start=True, stop=True)
                        ys = yT[hp][64 * h:64 * h + 64, q0:q0 + 512]
                        nc.vector.tensor_copy(ys, yh[h][0:64, :])
                        nc.vector.tensor_mul(ys, ys, rb[:])

        # ---- Phase 4: partial proj out[s, e] = sum_y yT[y, s] wproj[y, e] --
        with tc.tile_pool(name="outp", bufs=2) as outp:
            for st in range(NT):
                pa = ps_mm.tile([128, 512], F32, tag="mm", name="pa")
                pb = ps_mm.tile([128, 256], F32, tag="mm", name="pb")
                for yt in range(3):
                    nc.tensor.matmul(
                        pa[:], r(yT[yt][:, 128 * st:128 * st + 128]),
                        r(wp[yt][:, 0:512]),
                        start=(yt == 0), stop=(yt == 2))
                for yt in range(3):
                    nc.tensor.matmul(
                        pb[:], r(yT[yt][:, 128 * st:128 * st + 128]),
                        r(wp[yt][:, 512:768]),
                        start=(yt == 0), stop=(yt == 2))
                ot = outp.tile([128, D], F32, tag="ot", name="ot")
                nc.vector.tensor_copy(ot[:, 0:512], pa[:])
                nc.vector.tensor_copy(ot[:, 512:768], pb[:])
                nc.sync.dma_start(out=out[128 * st:128 * st + 128, :],
                                  in_=ot[:])

    nc.finalize()
    return nc


def round_fp32r(a):
    """Round fp32 to fp32r (11 explicit mantissa bits; low 12 bits zero),
    round-to-nearest-even, matching the PE's fp32r input format."""
    a = np.ascontiguousarray(a, dtype=np.float32)
    u = a.view(np.uint32).astype(np.uint64)
    bias = ((u >> 12) & 1) + 0x7FF
    u = ((u + bias) & 0xFFFFF000).astype(np.uint32)
    return u.view(np.float32)


def shard_inputs(x, w_qkv, b_qkv, w_proj):
    """Host-side sharding: returns list of per-core input dicts."""
    in_maps = []
    for core in range(NCORES):
        b, hg = (core // 2) % x.shape[0], core % 2
        cs = slice(384 * hg, 384 * hg + 384)
        xT_s = np.ascontiguousarray(x[b].T).astype(np.float32)
        wqkv_s = np.ascontiguousarray(np.concatenate(
            [w_qkv[:, 0:768][:, cs], w_qkv[:, 768:1536][:, cs],
             w_qkv[:, 1536:2304][:, cs]], axis=1))
        bqk = np.concatenate([b_qkv[0:768][cs], b_qkv[768:1536][cs]])
        bqk_s = np.ascontiguousarray(bqk.reshape(6, 128).T)
        bv_s = np.ascontiguousarray(b_qkv[1536:2304][cs].reshape(1, 384))
        wproj_s = np.ascontiguousarray(w_proj[384 * hg:384 * hg + 384, :])
        in_maps.append({
            "xT_s": round_fp32r(xT_s),
            "wqkv_s": round_fp32r(wqkv_s),
            "bqk_s": bqk_s.astype(np.float32),
            "bv_s": round_fp32r(bv_s),
            "wproj_s": round_fp32r(wproj_s),
        })
    return in_maps


_CACHED = {}


def _get_program():
    if "nc" not in _CACHED:
        _CACHED["nc"] = build_program()
    return _CACHED["nc"]


def kernel(x, w_qkv, b_qkv, w_proj, b_proj):
    from concourse.bass_utils import run_bass_kernel_spmd

    x = np.asarray(x, dtype=np.float32)
    w_qkv = np.asarray(w_qkv, dtype=np.float32)
    b_qkv = np.asarray(b_qkv, dtype=np.float32)
    w_proj = np.asarray(w_proj, dtype=np.float32)
    b_proj = np.asarray(b_proj, dtype=np.float32)

    B, S, dim = x.shape
    nc = _get_program()
    in_maps = shard_inputs(x, w_qkv, b_qkv, w_proj)
    res = run_bass_kernel_spmd(nc, in_maps, core_ids=list(range(NCORES)))
    parts = [m["out_s"] for m in res.results]
    outp = np.empty((B, S, dim), dtype=np.float32)
    for b in range(B):
        outp[b] = parts[2 * b] + parts[2 * b + 1] + b_proj[None, :]
    return outp


# revision 16
# speedup vs baseline: 1.1199x; 1.1199x over previous
"""Causal self-attention (B=4, S=2048, D=768, H=12) on 8 TRN2 NeuronCores.

Sharding: core = (batch b in 0..3) x (head-group hg in 0..1, 6 heads each).
Host pre-transposes x -> xT per batch, slices w_qkv columns / w_proj rows per
head-group.  Each core computes its 6 heads end-to-end and a partial
projection output [S, D]; the host sums the two head-group partials per batch
and adds b_proj.

Device layouts (per core):
  xT   [768, 2048]   (d on partitions)  -> 6 sbuf tiles [128, S]
  qkT  [768(qk cols), S]: rows 0-383 = qT (6 heads x 64), 384-767 = kT.
       6 tiles [128, S]; tile hp (0-2) = qT of head pair hp, tile 3+hp = kT.
  v    natural [S, 6, 65]: per s-tile [128, 6, 65]; col 64 of each head block
       is 1.0 -> the attn @ [v|1] matmul emits the softmax denominator row.
  scores computed TRANSPOSED: sT[kpos, qpos] = k . q  (lhsT=kT, rhs=qT,
       row-tiled pair: head0 at partitions 0-63, head1 at 64-127 run
       concurrently in the PE array).  Softmax denom = row 64 of yT psum.
  yT   [128 (pair y-dims), S] per pair -> proj lhsT directly.
"""

import numpy as np
from contextlib import ExitStack

import concourse.bass as bass
import concourse.bacc as bacc
import concourse.mybir as mybir
from concourse.tile import TileContext

F32 = mybir.dt.float32
F32R = mybir.dt.float32r

D = 768
NCORES = 8
SCALE = 0.125  # 1/sqrt(64)


def build_program(S=2048, use_f32r=True):
    NS = S // 512   # q strips
    NT = S // 128   # s tiles
    DT = D // 128   # d tiles (contraction)

    nc = bacc.Bacc()

    MDT = F32R if use_f32r else F32  # matmul input dtype

    xT = nc.dram_tensor("xT_s", [D, S], MDT, kind="ExternalInput")
    wqkv = nc.dram_tensor("wqkv_s", [D, 1152], MDT, kind="ExternalInput")
    bqk = nc.dram_tensor("bqk_s", [128, 6], F32, kind="ExternalInput")
    bv = nc.dram_tensor("bv_s", [1, 384], MDT, kind="ExternalInput")
    wproj = nc.dram_tensor("wproj_s", [384, D], MDT, kind="ExternalInput")
    out = nc.dram_tensor("out_s", [S, D], F32, kind="ExternalOutput")

    def r(ap):
        return ap

    with TileContext(nc) as tc, ExitStack() as ctx:
        persist = ctx.enter_context(tc.tile_pool(name="persist", bufs=1))

        qkT = [persist.tile([128, S], MDT, tag=f"qkT{i}", name=f"qkT{i}")
               for i in range(6)]
        v_sb = [persist.tile([128, 6, 65], MDT, tag=f"v{i}", name=f"v{i}")
                for i in range(NT)]
        yT = [persist.tile([128, S], MDT, tag=f"yT{i}", name=f"yT{i}")
              for i in range(3)]
        wp = [persist.tile([128, D], MDT, tag=f"wp{i}", name=f"wp{i}")
              for i in range(3)]
        bqk_sb = persist.tile([128, 6], F32, tag="bqk", name="bqk_sb")
        bv_sb = persist.tile([1, 384], MDT, tag="bv", name="bv_sb")
        ones = persist.tile([1, 128], MDT, tag="ones", name="ones_sb")
        ones_f = persist.tile([1, 64], F32, tag="ones_f", name="ones_f_sb")

        nc.vector.memset(ones[:].bitcast(F32), 1.0)
        nc.vector.memset(ones_f[:], 1.0)
        nc.sync.dma_start(out=bqk_sb[:], in_=bqk[:])
        nc.sync.dma_start(out=bv_sb[:], in_=bv[:])
        for i in range(3):
            nc.sync.dma_start(out=wp[i][:], in_=wproj[128 * i:128 * (i + 1), :])
        for st in range(NT):
            nc.vector.memset(v_sb[st][:, :, 64:65].bitcast(F32), 1.0)

        with tc.tile_pool(name="xw", bufs=1) as xw_pool, \
             tc.tile_pool(name="ps1", bufs=3, space="PSUM") as ps1:

            def pe_touch(ap):
                # Tiny self-matmul that makes the PE wait on this tile's
                # producer once, so later real matmuls carry at most ONE sync
                # wait each (fp32r self-loading matmul has 1 LW wait slot).
                t = ps1.tile([1, 1], F32, tag="mm", name="touch")
                nc.tensor.matmul(t[:], ap.bitcast(F32), ap.bitcast(F32),
                                 start=True, stop=True)

            xT_sb = [xw_pool.tile([128, S], MDT, tag=f"xT{i}", name=f"xTs{i}")
                     for i in range(DT)]
            w_sb = [xw_pool.tile([128, 1152], MDT, tag=f"w{i}", name=f"ws{i}")
                    for i in range(DT)]
            for i in range(DT):
                nc.sync.dma_start(out=xT_sb[i][:],
                                  in_=xT[128 * i:128 * (i + 1), :])
                nc.sync.dma_start(out=w_sb[i][:],
                                  in_=wqkv[128 * i:128 * (i + 1), :])
                pe_touch(xT_sb[i][:, 0:1])
                pe_touch(w_sb[i][:, 0:1])
            for i in range(3):
                pe_touch(wp[i][:, 0:1])

            # ---- Phase 1: qkT[c, s] = sum_d wqkv[d, c] * xT[d, s] + bias ----
            for ns in range(NS):
                for ct in range(6):
                    ps = ps1.tile([128, 512], F32, tag="mm", name="ps_qk")
                    for dt_i in range(DT):
                        nc.tensor.matmul(
                            ps[:],
                            r(w_sb[dt_i][:, 128 * ct:128 * ct + 128]),
                            r(xT_sb[dt_i][:, 512 * ns:512 * ns + 512]),
                            start=(dt_i == 0), stop=(dt_i == DT - 1))
                    nc.vector.tensor_scalar_add(
                        qkT[ct][:, 512 * ns:512 * ns + 512], ps[:],
                        bqk_sb[:, ct:ct + 1])

            # ---- Phase 2: v[s, c] = sum_d xT[d, s] * wv[d, c] + bv ----
            for st in range(NT):
                ps = ps1.tile([128, 384], F32, tag="mm", name="ps_v")
                for dt_i in range(DT):
                    nc.tensor.matmul(
                        ps[:],
                        r(xT_sb[dt_i][:, 128 * st:128 * st + 128]),
                        r(w_sb[dt_i][:, 768:1152]),
                        start=(dt_i == 0), stop=False)
                nc.tensor.matmul(ps[:], r(ones[:, 0:128]), r(bv_sb[:]),
                                 start=False, stop=True)
                nc.vector.tensor_copy(
                    v_sb[st][:, :, 0:64],
                    ps[:].rearrange("p (h e) -> p h e", h=6))
                pe_touch(v_sb[st][:, 0, 0:1])

        # ---- Phase 3: attention, scores transposed, per head pair ----
        with tc.tile_pool(name="ps_s", bufs=6, space="PSUM") as ps_s, \
             tc.tile_pool(name="ps_y", bufs=2, space="PSUM") as ps_y, \
             tc.tile_pool(name="expp", bufs=8) as expp, \
             tc.tile_pool(name="exp2p", bufs=6) as exp2p, \
             tc.tile_pool(name="rcp", bufs=4) as rcp:
            for ns in range(NS):
                q0 = 512 * ns
                for hp in range(3):
                    qt = qkT[hp]
                    kt = qkT[3 + hp]
                    nk = 4 * (ns + 1)
                    yh = [ps_y.tile([65, 512], F32, tag="yh", name="yh0"),
                          ps_y.tile([65, 512], F32, tag="yh", name="yh1")]
                    for kb in range(nk):
                        diag = kb >= 4 * ns
                        c0 = 128 * kb - q0 if diag else 0
                        for h in range(2):
                            p0 = 64 * h
                            sc = ps_s.tile([128, 512], F32, tag="sc",
                                           name="sc")
                            nc.tensor.matmul(
                                sc[:, c0:512],
                                r(kt[p0:p0 + 64, 128 * kb:128 * kb + 128]),
                                r(qt[p0:p0 + 64, q0 + c0:q0 + 512]),
                                start=True, stop=True)
                            ex = expp.tile([128, 512], MDT, tag="exp",
                                           name="ex")
                            nc.scalar.activation(
                                ex[:, c0:512], sc[:, c0:512],
                                mybir.ActivationFunctionType.Exp, scale=SCALE)
                            if diag:
                                # causal triangle -> 0, into a fresh tile so
                                # the yT matmul depends on Pool only
                                ex2 = exp2p.tile([128, 512], MDT,
                                                 tag="exp2", name="ex2")
                                nc.gpsimd.affine_select(
                                    out=ex2[:, c0:512],
                                    in_=ex[:, c0:512],
                                    compare_op=mybir.AluOpType.is_ge,
                                    fill=0.0, base=0,
                                    pattern=[[1, 512 - c0]],
                                    channel_multiplier=-1)
                                ex = ex2
                            nc.tensor.matmul(
                                yh[h][:, c0:512],
                                r(v_sb[kb][:, 2 * hp + h, :]),
                                r(ex[:, c0:512]),
                                start=(kb == 0), stop=(kb == nk - 1),
                                skip_group_check=True)
                    for h in range(2):
                        lrow = rcp.tile([1, 512], F32, tag="lrow",
                                        name="lrow")
                        nc.vector.tensor_copy(lrow[:], yh[h][64:65, :])
                        rec = rcp.tile([1, 512], F32, tag="rec", name="rec")
                        nc.vector.reciprocal_approx_fast(rec[:], lrow[:])
                        rb = ps_s.tile([64, 512], F32, tag="sc", name="rb")
                        nc.tensor.matmul(rb[:], ones_f[:, 0:64], rec[:],
                                         start=True, stop=True)
                        ys = yT[hp][64 * h:64 * h + 64, q0:q0 + 512]
                        nc.vector.tensor_copy(ys, yh[h][0:64, :])
                        nc.vector.tensor_mul(ys, ys, rb[:])

        # ---- Phase 4: partial proj out[s, e] = sum_y yT[y, s] wproj[y, e] --
        with tc.tile_pool(name="ps_o", bufs=2, space="PSUM") as ps_o, \
             tc.tile_pool(name="outp", bufs=2) as outp:
            for st in range(NT):
                pa = ps_o.tile([128, 512], F32, tag="pa", name="pa")
                pb = ps_o.tile([128, 256], F32, tag="pb", name="pb")
                for yt in range(3):
                    nc.tensor.matmul(
                        pa[:], r(yT[yt][:, 128 * st:128 * st + 128]),
                        r(wp[yt][:, 0:512]),
                        start=(yt == 0), stop=(yt == 2))
                for yt in range(3):
                    nc.tensor.matmul(
                        pb[:], r(yT[yt][:, 128 * st:128 * st + 128]),
                        r(wp[yt][:, 512:768]),
                        start=(yt == 0), stop=(yt == 2))
                ot = outp.tile([128, D], F32, tag="ot", name="ot")
                nc.vector.tensor_copy(ot[:, 0:512], pa[:])
                nc.vector.tensor_copy(ot[:, 512:768], pb[:])
                nc.sync.dma_start(out=out[128 * st:128 * st + 128, :],
                                  in_=ot[:])

    nc.finalize()
    return nc


def round_fp32r(a):
    """Round fp32 to fp32r (11 explicit mantissa bits; low 12 bits zero),
    round-to-nearest-even, matching the PE's fp32r input format."""
    a = np.ascontiguousarray(a, dtype=np.float32)
    u = a.view(np.uint32).astype(np.uint64)
    bias = ((u >> 12) & 1) + 0x7FF
    u = ((u + bias) & 0xFFFFF000).astype(np.uint32)
    return u.view(np.float32)


def shard_inputs(x, w_qkv, b_qkv, w_proj):
    """Host-side sharding: returns list of per-core input dicts."""
    in_maps = []
    for core in range(NCORES):
        b, hg = (core // 2) % x.shape[0], core % 2
        cs = slice(384 * hg, 384 * hg + 384)
        xT_s = np.ascontiguousarray(x[b].T).astype(np.float32)
        wqkv_s = np.ascontiguousarray(np.concatenate(
            [w_qkv[:, 0:768][:, cs], w_qkv[:, 768:1536][:, cs],
             w_qkv[:, 1536:2304][:, cs]], axis=1))
        bqk = np.concatenate([b_qkv[0:768][cs], b_qkv[768:1536][cs]])
        bqk_s = np.ascontiguousarray(bqk.reshape(6, 128).T)
        bv_s = np.ascontiguousarray(b_qkv[1536:2304][cs].reshape(1, 384))
        wproj_s = np.ascontiguousarray(w_proj[384 * hg:384 * hg + 384, :])
        in_maps.append({
            "xT_s": round_fp32r(xT_s),
            "wqkv_s": round_fp32r(wqkv_s),
            "bqk_s": bqk_s.astype(np.float32),
            "bv_s": round_fp32r(bv_s),
            "wproj_s": round_fp32r(wproj_s),
        })
    return in_maps


_CACHED = {}


def _get_program():
    if "nc" not in _CACHED:
        _CACHED["nc"] = build_program()
    return _CACHED["nc"]


def kernel(x, w_qkv, b_qkv, w_proj, b_proj):
    from concourse.bass_utils import run_bass_kernel_spmd

    x = np.asarray(x, dtype=np.float32)
    w_qkv = np.asarray(w_qkv, dtype=np.float32)
    b_qkv = np.asarray(b_qkv, dtype=np.float32)
    w_proj = np.asarray(w_proj, dtype=np.float32)
    b_proj = np.asarray(b_proj, dtype=np.float32)

    B, S, dim = x.shape
    nc = _get_program()
    in_maps = shard_inputs(x, w_qkv, b_qkv, w_proj)
    res = run_bass_kernel_spmd(nc, in_maps, core_ids=list(range(NCORES)))
    parts = [m["out_s"] for m in res.results]
    outp = np.empty((B, S, dim), dtype=np.float32)
    for b in range(B):
        outp[b] = parts[2 * b] + parts[2 * b + 1] + b_proj[None, :]
    return outp


# revision 17
# speedup vs baseline: 1.2041x; 1.0752x over previous
"""Causal self-attention (B=4, S=2048, D=768, H=12) on 8 TRN2 NeuronCores.

Sharding: core = (batch b in 0..3) x (head-group hg in 0..1, 6 heads each).
Host pre-transposes x -> xT per batch, slices w_qkv columns / w_proj rows per
head-group.  Each core computes its 6 heads end-to-end and a partial
projection output [S, D]; the host sums the two head-group partials per batch
and adds b_proj.

Device layouts (per core):
  xT   [768, 2048]   (d on partitions)  -> 6 sbuf tiles [128, S]
  qkT  [768(qk cols), S]: rows 0-383 = qT (6 heads x 64), 384-767 = kT.
       6 tiles [128, S]; tile hp (0-2) = qT of head pair hp, tile 3+hp = kT.
  v    natural [S, 6, 65]: per s-tile [128, 6, 65]; col 64 of each head block
       is 1.0 -> the attn @ [v|1] matmul emits the softmax denominator row.
  scores computed TRANSPOSED: sT[kpos, qpos] = k . q  (lhsT=kT, rhs=qT,
       row-tiled pair: head0 at partitions 0-63, head1 at 64-127 run
       concurrently in the PE array).  Softmax denom = row 64 of yT psum.
  yT   [128 (pair y-dims), S] per pair -> proj lhsT directly.
"""

import numpy as np
from contextlib import ExitStack

import concourse.bass as bass
import concourse.bacc as bacc
import concourse.mybir as mybir
from concourse.tile import TileContext

F32 = mybir.dt.float32
F32R = mybir.dt.float32r
BF16 = mybir.dt.bfloat16

D = 768
NCORES = 8
SCALE = 0.125  # 1/sqrt(64)


def build_program(S=2048, use_f32r=True):
    NS = S // 512   # q strips
    NT = S // 128   # s tiles
    DT = D // 128   # d tiles (contraction)

    nc = bacc.Bacc()

    MDT = F32R if use_f32r else F32  # matmul input dtype

    xT = nc.dram_tensor("xT_s", [D, S], MDT, kind="ExternalInput")
    wqkv = nc.dram_tensor("wqkv_s", [D, 1152], MDT, kind="ExternalInput")
    bqk = nc.dram_tensor("bqk_s", [128, 6], F32, kind="ExternalInput")
    bv = nc.dram_tensor("bv_s", [1, 384], MDT, kind="ExternalInput")
    wproj = nc.dram_tensor("wproj_s", [384, D], MDT, kind="ExternalInput")
    out = nc.dram_tensor("out_s", [S, D], F32, kind="ExternalOutput")

    def r(ap):
        return ap

    with TileContext(nc) as tc, ExitStack() as ctx:
        persist = ctx.enter_context(tc.tile_pool(name="persist", bufs=1))

        qkT = [persist.tile([128, S], BF16, tag=f"qkT{i}", name=f"qkT{i}")
               for i in range(6)]
        v_sb = [persist.tile([128, 6, 65], MDT, tag=f"v{i}", name=f"v{i}")
                for i in range(NT)]
        yT = [persist.tile([128, S], MDT, tag=f"yT{i}", name=f"yT{i}")
              for i in range(3)]
        wp = [persist.tile([128, D], MDT, tag=f"wp{i}", name=f"wp{i}")
              for i in range(3)]
        bqk_sb = persist.tile([128, 6], F32, tag="bqk", name="bqk_sb")
        bv_sb = persist.tile([1, 384], MDT, tag="bv", name="bv_sb")
        ones = persist.tile([1, 128], MDT, tag="ones", name="ones_sb")
        ones_f = persist.tile([1, 64], F32, tag="ones_f", name="ones_f_sb")

        mask = persist.tile([128, 512], MDT, tag="mask", name="mask_sb")
        nc.vector.memset(ones[:].bitcast(F32), 1.0)
        nc.vector.memset(ones_f[:], 1.0)
        nc.vector.memset(mask[:].bitcast(F32), 1.0)
        # mask[p, j] = 1 if j >= p else 0 (cols 128+ stay 1)
        nc.gpsimd.affine_select(
            out=mask[:, 0:128], in_=mask[:, 0:128],
            compare_op=mybir.AluOpType.is_ge, fill=0.0, base=0,
            pattern=[[1, 128]], channel_multiplier=-1)
        nc.sync.dma_start(out=bqk_sb[:], in_=bqk[:])
        nc.sync.dma_start(out=bv_sb[:], in_=bv[:])
        for i in range(3):
            nc.sync.dma_start(out=wp[i][:], in_=wproj[128 * i:128 * (i + 1), :])
        for st in range(NT):
            nc.vector.memset(v_sb[st][:, :, 64:65].bitcast(F32), 1.0)

        with tc.tile_pool(name="xw", bufs=1) as xw_pool, \
             tc.tile_pool(name="ps1", bufs=3, space="PSUM") as ps1:

            def pe_touch(ap):
                # Tiny self-matmul that makes the PE wait on this tile's
                # producer once, so later real matmuls carry at most ONE sync
                # wait each (fp32r self-loading matmul has 1 LW wait slot).
                t = ps1.tile([1, 1], F32, tag="mm", name="touch")
                nc.tensor.matmul(t[:], ap.bitcast(F32), ap.bitcast(F32),
                                 start=True, stop=True)

            xT_sb = [xw_pool.tile([128, S], MDT, tag=f"xT{i}", name=f"xTs{i}")
                     for i in range(DT)]
            w_sb = [xw_pool.tile([128, 1152], MDT, tag=f"w{i}", name=f"ws{i}")
                    for i in range(DT)]
            for i in range(DT):
                nc.sync.dma_start(out=xT_sb[i][:],
                                  in_=xT[128 * i:128 * (i + 1), :])
                nc.sync.dma_start(out=w_sb[i][:],
                                  in_=wqkv[128 * i:128 * (i + 1), :])
                pe_touch(xT_sb[i][:, 0:1])
                pe_touch(w_sb[i][:, 0:1])
            for i in range(3):
                pe_touch(wp[i][:, 0:1])

            # ---- Phase 1: qkT[c, s] = sum_d wqkv[d, c] * xT[d, s] + bias ----
            for ns in range(NS):
                for ct in range(6):
                    ps = ps1.tile([128, 512], F32, tag="mm", name="ps_qk")
                    for dt_i in range(DT):
                        nc.tensor.matmul(
                            ps[:],
                            r(w_sb[dt_i][:, 128 * ct:128 * ct + 128]),
                            r(xT_sb[dt_i][:, 512 * ns:512 * ns + 512]),
                            start=(dt_i == 0), stop=(dt_i == DT - 1))
                    nc.vector.tensor_scalar_add(
                        qkT[ct][:, 512 * ns:512 * ns + 512], ps[:],
                        bqk_sb[:, ct:ct + 1])

            # ---- Phase 2: v[s, c] = sum_d xT[d, s] * wv[d, c] + bv ----
            for st in range(NT):
                ps = ps1.tile([128, 384], F32, tag="mm", name="ps_v")
                for dt_i in range(DT):
                    nc.tensor.matmul(
                        ps[:],
                        r(xT_sb[dt_i][:, 128 * st:128 * st + 128]),
                        r(w_sb[dt_i][:, 768:1152]),
                        start=(dt_i == 0), stop=False)
                nc.tensor.matmul(ps[:], r(ones[:, 0:128]), r(bv_sb[:]),
                                 start=False, stop=True)
                nc.vector.tensor_copy(
                    v_sb[st][:, :, 0:64],
                    ps[:].rearrange("p (h e) -> p h e", h=6))
                pe_touch(v_sb[st][:, 0, 0:1])

        # ---- Phase 3: attention, scores transposed, per head pair ----
        with tc.tile_pool(name="ps_s", bufs=6, space="PSUM") as ps_s, \
             tc.tile_pool(name="ps_y", bufs=2, space="PSUM") as ps_y, \
             tc.tile_pool(name="expp", bufs=8) as expp, \
             tc.tile_pool(name="exp2p", bufs=6) as exp2p, \
             tc.tile_pool(name="rcp", bufs=4) as rcp:
            for ns in range(NS):
                q0 = 512 * ns
                for hp in range(3):
                    qt = qkT[hp]
                    kt = qkT[3 + hp]
                    nk = 4 * (ns + 1)
                    yh = [ps_y.tile([65, 512], F32, tag="yh", name="yh0"),
                          ps_y.tile([65, 512], F32, tag="yh", name="yh1")]
                    for kb in range(nk):
                        diag = kb >= 4 * ns
                        c0 = 128 * kb - q0 if diag else 0
                        for h in range(2):
                            p0 = 64 * h
                            sc = ps_s.tile([128, 512], F32, tag="sc",
                                           name="sc")
                            nc.tensor.matmul(
                                sc[:, c0:512],
                                r(kt[p0:p0 + 64, 128 * kb:128 * kb + 128]),
                                r(qt[p0:p0 + 64, q0 + c0:q0 + 512]),
                                start=True, stop=True)
                            ex = expp.tile([128, 512], MDT, tag="exp",
                                           name="ex")
                            nc.scalar.activation(
                                ex[:, c0:512], sc[:, c0:512],
                                mybir.ActivationFunctionType.Exp, scale=SCALE)
                            if diag:
                                # causal triangle -> 0, via precomputed mask
                                # (DVE; fresh tile so the yT matmul sees a
                                # single-engine dependency)
                                ex2 = exp2p.tile([128, 512], MDT,
                                                 tag="exp2", name="ex2")
                                nc.vector.tensor_mul(ex2[:, c0:512],
                                                     ex[:, c0:512],
                                                     mask[:, 0:512 - c0])
                                ex = ex2
                            nc.tensor.matmul(
                                yh[h][:, c0:512],
                                r(v_sb[kb][:, 2 * hp + h, :]),
                                r(ex[:, c0:512]),
                                start=(kb == 0), stop=(kb == nk - 1),
                                skip_group_check=True)
                    for h in range(2):
                        lrow = rcp.tile([1, 512], F32, tag="lrow",
                                        name="lrow")
                        nc.vector.tensor_copy(lrow[:], yh[h][64:65, :])
                        rec = rcp.tile([1, 512], F32, tag="rec", name="rec")
                        nc.vector.reciprocal_approx_fast(rec[:], lrow[:])
                        rb = ps_s.tile([64, 512], F32, tag="sc", name="rb")
                        nc.tensor.matmul(rb[:], ones_f[:, 0:64], rec[:],
                                         start=True, stop=True)
                        ys = yT[hp][64 * h:64 * h + 64, q0:q0 + 512]
                        nc.vector.tensor_copy(ys, yh[h][0:64, :])
                        nc.vector.tensor_mul(ys, ys, rb[:])

        # ---- Phase 4: partial proj out[s, e] = sum_y yT[y, s] wproj[y, e] --
        with tc.tile_pool(name="ps_o", bufs=2, space="PSUM") as ps_o, \
             tc.tile_pool(name="outp", bufs=2) as outp:
            for st in range(NT):
                pa = ps_o.tile([128, 512], F32, tag="pa", name="pa")
                pb = ps_o.tile([128, 256], F32, tag="pb", name="pb")
                for yt in range(3):
                    nc.tensor.matmul(
                        pa[:], r(yT[yt][:, 128 * st:128 * st + 128]),
                        r(wp[yt][:, 0:512]),
                        start=(yt == 0), stop=(yt == 2))
                for yt in range(3):
                    nc.tensor.matmul(
                        pb[:], r(yT[yt][:, 128 * st:128 * st + 128]),
                        r(wp[yt][:, 512:768]),
                        start=(yt == 0), stop=(yt == 2))
                ot = outp.tile([128, D], F32, tag="ot", name="ot")
                nc.vector.tensor_copy(ot[:, 0:512], pa[:])
                nc.vector.tensor_copy(ot[:, 512:768], pb[:])
                nc.sync.dma_start(out=out[128 * st:128 * st + 128, :],
                                  in_=ot[:])

    nc.finalize()
    return nc


def round_fp32r(a):
    """Round fp32 to fp32r (11 explicit mantissa bits; low 12 bits zero),
    round-to-nearest-even, matching the PE's fp32r input format."""
    a = np.ascontiguousarray(a, dtype=np.float32)
    u = a.view(np.uint32).astype(np.uint64)
    bias = ((u >> 12) & 1) + 0x7FF
    u = ((u + bias) & 0xFFFFF000).astype(np.uint32)
    return u.view(np.float32)


def shard_inputs(x, w_qkv, b_qkv, w_proj):
    """Host-side sharding: returns list of per-core input dicts."""
    in_maps = []
    for core in range(NCORES):
        b, hg = (core // 2) % x.shape[0], core % 2
        cs = slice(384 * hg, 384 * hg + 384)
        xT_s = np.ascontiguousarray(x[b].T).astype(np.float32)
        wqkv_s = np.ascontiguousarray(np.concatenate(
            [w_qkv[:, 0:768][:, cs], w_qkv[:, 768:1536][:, cs],
             w_qkv[:, 1536:2304][:, cs]], axis=1))
        bqk = np.concatenate([b_qkv[0:768][cs], b_qkv[768:1536][cs]])
        bqk_s = np.ascontiguousarray(bqk.reshape(6, 128).T)
        bv_s = np.ascontiguousarray(b_qkv[1536:2304][cs].reshape(1, 384))
        wproj_s = np.ascontiguousarray(w_proj[384 * hg:384 * hg + 384, :])
        in_maps.append({
            "xT_s": round_fp32r(xT_s),
            "wqkv_s": round_fp32r(wqkv_s),
            "bqk_s": bqk_s.astype(np.float32),
            "bv_s": round_fp32r(bv_s),
            "wproj_s": round_fp32r(wproj_s),
        })
    return in_maps


_CACHED = {}


def _get_program():
    if "nc" not in _CACHED:
        _CACHED["nc"] = build_program()
    return _CACHED["nc"]


def kernel(x, w_qkv, b_qkv, w_proj, b_proj):
    from concourse.bass_utils import run_bass_kernel_spmd

    x = np.asarray(x, dtype=np.float32)
    w_qkv = np.asarray(w_qkv, dtype=np.float32)
    b_qkv = np.asarray(b_qkv, dtype=np.float32)
    w_proj = np.asarray(w_proj, dtype=np.float32)
    b_proj = np.asarray(b_proj, dtype=np.float32)

    B, S, dim = x.shape
    nc = _get_program()
    in_maps = shard_inputs(x, w_qkv, b_qkv, w_proj)
    res = run_bass_kernel_spmd(nc, in_maps, core_ids=list(range(NCORES)))
    parts = [m["out_s"] for m in res.results]
    outp = np.empty((B, S, dim), dtype=np.float32)
    for b in range(B):
        outp[b] = parts[2 * b] + parts[2 * b + 1] + b_proj[None, :]
    return outp


# revision 18
# speedup vs baseline: 1.2094x; 1.0044x over previous
"""Causal self-attention (B=4, S=2048, D=768, H=12) on 8 TRN2 NeuronCores.

Sharding: core = (batch b in 0..3) x (head-group hg in 0..1, 6 heads each).
Host pre-transposes x -> xT per batch, slices w_qkv columns / w_proj rows per
head-group.  Each core computes its 6 heads end-to-end and a partial
projection output [S, D]; the host sums the two head-group partials per batch
and adds b_proj.

Device layouts (per core):
  xT   [768, 2048]   (d on partitions)  -> 6 sbuf tiles [128, S]
  qkT  [768(qk cols), S]: rows 0-383 = qT (6 heads x 64), 384-767 = kT.
       6 tiles [128, S]; tile hp (0-2) = qT of head pair hp, tile 3+hp = kT.
  v    natural [S, 6, 65]: per s-tile [128, 6, 65]; col 64 of each head block
       is 1.0 -> the attn @ [v|1] matmul emits the softmax denominator row.
  scores computed TRANSPOSED: sT[kpos, qpos] = k . q  (lhsT=kT, rhs=qT,
       row-tiled pair: head0 at partitions 0-63, head1 at 64-127 run
       concurrently in the PE array).  Softmax denom = row 64 of yT psum.
  yT   [128 (pair y-dims), S] per pair -> proj lhsT directly.
"""

import numpy as np
from contextlib import ExitStack

import concourse.bass as bass
import concourse.bacc as bacc
import concourse.mybir as mybir
from concourse.tile import TileContext

F32 = mybir.dt.float32
F32R = mybir.dt.float32r
BF16 = mybir.dt.bfloat16

D = 768
NCORES = 8
SCALE = 0.125  # 1/sqrt(64)


def build_program(S=2048, use_f32r=True):
    NS = S // 512   # q strips
    NT = S // 128   # s tiles
    DT = D // 128   # d tiles (contraction)

    nc = bacc.Bacc()

    MDT = F32R if use_f32r else F32  # matmul input dtype

    xT = nc.dram_tensor("xT_s", [D, S], MDT, kind="ExternalInput")
    wqkv = nc.dram_tensor("wqkv_s", [D, 1152], MDT, kind="ExternalInput")
    bqk = nc.dram_tensor("bqk_s", [128, 6], F32, kind="ExternalInput")
    bv = nc.dram_tensor("bv_s", [1, 384], MDT, kind="ExternalInput")
    wproj = nc.dram_tensor("wproj_s", [384, D], MDT, kind="ExternalInput")
    out = nc.dram_tensor("out_s", [S, D], F32, kind="ExternalOutput")

    def r(ap):
        return ap

    with TileContext(nc) as tc, ExitStack() as ctx:
        persist = ctx.enter_context(tc.tile_pool(name="persist", bufs=1))

        qkT = [persist.tile([128, S], BF16, tag=f"qkT{i}", name=f"qkT{i}")
               for i in range(6)]
        v_sb = [persist.tile([128, 6, 65], MDT, tag=f"v{i}", name=f"v{i}")
                for i in range(NT)]
        yT = [persist.tile([128, S], MDT, tag=f"yT{i}", name=f"yT{i}")
              for i in range(3)]
        wp = [persist.tile([128, D], MDT, tag=f"wp{i}", name=f"wp{i}")
              for i in range(3)]
        bqk_sb = persist.tile([128, 6], F32, tag="bqk", name="bqk_sb")
        bv_sb = persist.tile([1, 384], MDT, tag="bv", name="bv_sb")
        ones = persist.tile([1, 128], MDT, tag="ones", name="ones_sb")
        ones_f = persist.tile([1, 64], F32, tag="ones_f", name="ones_f_sb")

        maskb = persist.tile([128, 512], F32, tag="maskb", name="maskb_sb")
        nc.vector.memset(ones[:].bitcast(F32), 1.0)
        nc.vector.memset(ones_f[:], 1.0)
        nc.vector.memset(maskb[:], 0.0)
        # maskb[p, j] = 0 if j >= p else -30 (cols 128+ stay 0); adding it to
        # the scaled scores before exp() sends masked entries to ~1e-13
        nc.gpsimd.affine_select(
            out=maskb[:, 0:128], in_=maskb[:, 0:128],
            compare_op=mybir.AluOpType.is_ge, fill=-30.0, base=0,
            pattern=[[1, 128]], channel_multiplier=-1)
        nc.sync.dma_start(out=bqk_sb[:], in_=bqk[:])
        nc.sync.dma_start(out=bv_sb[:], in_=bv[:])
        for i in range(3):
            nc.sync.dma_start(out=wp[i][:], in_=wproj[128 * i:128 * (i + 1), :])
        for st in range(NT):
            nc.vector.memset(v_sb[st][:, :, 64:65].bitcast(F32), 1.0)

        with tc.tile_pool(name="xw", bufs=1) as xw_pool, \
             tc.tile_pool(name="ps1", bufs=3, space="PSUM") as ps1:

            def pe_touch(ap):
                # Tiny self-matmul that makes the PE wait on this tile's
                # producer once, so later real matmuls carry at most ONE sync
                # wait each (fp32r self-loading matmul has 1 LW wait slot).
                t = ps1.tile([1, 1], F32, tag="mm", name="touch")
                nc.tensor.matmul(t[:], ap.bitcast(F32), ap.bitcast(F32),
                                 start=True, stop=True)

            xT_sb = [xw_pool.tile([128, S], MDT, tag=f"xT{i}", name=f"xTs{i}")
                     for i in range(DT)]
            w_sb = [xw_pool.tile([128, 1152], MDT, tag=f"w{i}", name=f"ws{i}")
                    for i in range(DT)]
            for i in range(DT):
                nc.sync.dma_start(out=xT_sb[i][:],
                                  in_=xT[128 * i:128 * (i + 1), :])
                nc.sync.dma_start(out=w_sb[i][:],
                                  in_=wqkv[128 * i:128 * (i + 1), :])
                pe_touch(xT_sb[i][:, 0:1])
                pe_touch(w_sb[i][:, 0:1])
            for i in range(3):
                pe_touch(wp[i][:, 0:1])

            # ---- Phase 1: qkT[c, s] = sum_d wqkv[d, c] * xT[d, s] + bias ----
            for ns in range(NS):
                for ct in range(6):
                    ps = ps1.tile([128, 512], F32, tag="mm", name="ps_qk")
                    for dt_i in range(DT):
                        nc.tensor.matmul(
                            ps[:],
                            r(w_sb[dt_i][:, 128 * ct:128 * ct + 128]),
                            r(xT_sb[dt_i][:, 512 * ns:512 * ns + 512]),
                            start=(dt_i == 0), stop=(dt_i == DT - 1))
                    nc.vector.tensor_scalar_add(
                        qkT[ct][:, 512 * ns:512 * ns + 512], ps[:],
                        bqk_sb[:, ct:ct + 1])

            # ---- Phase 2: v[s, c] = sum_d xT[d, s] * wv[d, c] + bv ----
            for st in range(NT):
                ps = ps1.tile([128, 384], F32, tag="mm", name="ps_v")
                for dt_i in range(DT):
                    nc.tensor.matmul(
                        ps[:],
                        r(xT_sb[dt_i][:, 128 * st:128 * st + 128]),
                        r(w_sb[dt_i][:, 768:1152]),
                        start=(dt_i == 0), stop=False)
                nc.tensor.matmul(ps[:], r(ones[:, 0:128]), r(bv_sb[:]),
                                 start=False, stop=True)
                nc.vector.tensor_copy(
                    v_sb[st][:, :, 0:64],
                    ps[:].rearrange("p (h e) -> p h e", h=6))
                pe_touch(v_sb[st][:, 0, 0:1])

        # ---- Phase 3: attention, scores transposed, per head pair ----
        with tc.tile_pool(name="ps_s", bufs=6, space="PSUM") as ps_s, \
             tc.tile_pool(name="ps_y", bufs=2, space="PSUM") as ps_y, \
             tc.tile_pool(name="expp", bufs=8) as expp, \
             tc.tile_pool(name="exp2p", bufs=6) as exp2p, \
             tc.tile_pool(name="rcp", bufs=4) as rcp:
            for ns in range(NS):
                q0 = 512 * ns
                for hp in range(3):
                    qt = qkT[hp]
                    kt = qkT[3 + hp]
                    nk = 4 * (ns + 1)
                    yh = [ps_y.tile([65, 512], F32, tag="yh", name="yh0"),
                          ps_y.tile([65, 512], F32, tag="yh", name="yh1")]
                    for kb in range(nk):
                        diag = kb >= 4 * ns
                        c0 = 128 * kb - q0 if diag else 0
                        for h in range(2):
                            p0 = 64 * h
                            sc = ps_s.tile([128, 512], F32, tag="sc",
                                           name="sc")
                            nc.tensor.matmul(
                                sc[:, c0:512],
                                r(kt[p0:p0 + 64, 128 * kb:128 * kb + 128]),
                                r(qt[p0:p0 + 64, q0 + c0:q0 + 512]),
                                start=True, stop=True)
                            ex = expp.tile([128, 512], MDT, tag="exp",
                                           name="ex")
                            if diag:
                                # causal mask: scaled scores + (-30) above
                                # the diagonal, on DVE, then exp on ACT.
                                # Keeps every hop single-wait: PE->DVE->ACT.
                                sm = exp2p.tile([128, 512], F32, tag="sm",
                                                name="sm")
                                nc.vector.scalar_tensor_tensor(
                                    sm[:, c0:512], sc[:, c0:512], SCALE,
                                    maskb[:, 0:512 - c0],
                                    op0=mybir.AluOpType.mult,
                                    op1=mybir.AluOpType.add)
                                nc.scalar.activation(
                                    ex[:, c0:512], sm[:, c0:512],
                                    mybir.ActivationFunctionType.Exp,
                                    scale=1.0)
                            else:
                                nc.scalar.activation(
                                    ex[:, c0:512], sc[:, c0:512],
                                    mybir.ActivationFunctionType.Exp,
                                    scale=SCALE)
                            nc.tensor.matmul(
                                yh[h][:, c0:512],
                                r(v_sb[kb][:, 2 * hp + h, :]),
                                r(ex[:, c0:512]),
                                start=(kb == 0), stop=(kb == nk - 1),
                                skip_group_check=True)
                    for h in range(2):
                        lrow = rcp.tile([1, 512], F32, tag="lrow",
                                        name="lrow")
                        nc.vector.tensor_copy(lrow[:], yh[h][64:65, :])
                        rec = rcp.tile([1, 512], F32, tag="rec", name="rec")
                        nc.vector.reciprocal_approx_fast(rec[:], lrow[:])
                        rb = ps_s.tile([64, 512], F32, tag="sc", name="rb")
                        nc.tensor.matmul(rb[:], ones_f[:, 0:64], rec[:],
                                         start=True, stop=True)
                        ys = yT[hp][64 * h:64 * h + 64, q0:q0 + 512]
                        nc.vector.tensor_copy(ys, yh[h][0:64, :])
                        nc.vector.tensor_mul(ys, ys, rb[:])

        # ---- Phase 4: partial proj out[s, e] = sum_y yT[y, s] wproj[y, e] --
        with tc.tile_pool(name="ps_o", bufs=2, space="PSUM") as ps_o, \
             tc.tile_pool(name="outp", bufs=2) as outp:
            for st in range(NT):
                pa = ps_o.tile([128, 512], F32, tag="pa", name="pa")
                pb = ps_o.tile([128, 256], F32, tag="pb", name="pb")
                for yt in range(3):
                    nc.tensor.matmul(
                        pa[:], r(yT[yt][:, 128 * st:128 * st + 128]),
                        r(wp[yt][:, 0:512]),
                        start=(yt == 0), stop=(yt == 2))
                for yt in range(3):
                    nc.tensor.matmul(
                        pb[:], r(yT[yt][:, 128 * st:128 * st + 128]),
                        r(wp[yt][:, 512:768]),
                        start=(yt == 0), stop=(yt == 2))
                ot = outp.tile([128, D], F32, tag="ot", name="ot")
                nc.vector.tensor_copy(ot[:, 0:512], pa[:])
                nc.vector.tensor_copy(ot[:, 512:768], pb[:])
                nc.sync.dma_start(out=out[128 * st:128 * st + 128, :],
                                  in_=ot[:])

    nc.finalize()
    return nc


def round_fp32r(a):
    """Round fp32 to fp32r (11 explicit mantissa bits; low 12 bits zero),
    round-to-nearest-even, matching the PE's fp32r input format."""
    a = np.ascontiguousarray(a, dtype=np.float32)
    u = a.view(np.uint32).astype(np.uint64)
    bias = ((u >> 12) & 1) + 0x7FF
    u = ((u + bias) & 0xFFFFF000).astype(np.uint32)
    return u.view(np.float32)


def shard_inputs(x, w_qkv, b_qkv, w_proj):
    """Host-side sharding: returns list of per-core input dicts."""
    in_maps = []
    for core in range(NCORES):
        b, hg = (core // 2) % x.shape[0], core % 2
        cs = slice(384 * hg, 384 * hg + 384)
        xT_s = np.ascontiguousarray(x[b].T).astype(np.float32)
        wqkv_s = np.ascontiguousarray(np.concatenate(
            [w_qkv[:, 0:768][:, cs], w_qkv[:, 768:1536][:, cs],
             w_qkv[:, 1536:2304][:, cs]], axis=1))
        bqk = np.concatenate([b_qkv[0:768][cs], b_qkv[768:1536][cs]])
        bqk_s = np.ascontiguousarray(bqk.reshape(6, 128).T)
        bv_s = np.ascontiguousarray(b_qkv[1536:2304][cs].reshape(1, 384))
        wproj_s = np.ascontiguousarray(w_proj[384 * hg:384 * hg + 384, :])
        in_maps.append({
            "xT_s": round_fp32r(xT_s),
            "wqkv_s": round_fp32r(wqkv_s),
            "bqk_s": bqk_s.astype(np.float32),
            "bv_s": round_fp32r(bv_s),
            "wproj_s": round_fp32r(wproj_s),
        })
    return in_maps


_CACHED = {}


def _get_program():
    if "nc" not in _CACHED:
        _CACHED["nc"] = build_program()
    return _CACHED["nc"]


def kernel(x, w_qkv, b_qkv, w_proj, b_proj):
    from concourse.bass_utils import run_bass_kernel_spmd

    x = np.asarray(x, dtype=np.float32)
    w_qkv = np.asarray(w_qkv, dtype=np.float32)
    b_qkv = np.asarray(b_qkv, dtype=np.float32)
    w_proj = np.asarray(w_proj, dtype=np.float32)
    b_proj = np.asarray(b_proj, dtype=np.float32)

    B, S, dim = x.shape
    nc = _get_program()
    in_maps = shard_inputs(x, w_qkv, b_qkv, w_proj)
    res = run_bass_kernel_spmd(nc, in_maps, core_ids=list(range(NCORES)))
    parts = [m["out_s"] for m in res.results]
    outp = np.empty((B, S, dim), dtype=np.float32)
    for b in range(B):
        outp[b] = parts[2 * b] + parts[2 * b + 1] + b_proj[None, :]
    return outp


# revision 21
# speedup vs baseline: 1.4888x; 1.2310x over previous
"""Causal self-attention (B=4, S=2048, D=768, H=12) on 8 TRN2 NeuronCores.

Sharding: core = (batch b in 0..3) x (head-group hg in 0..1, 6 heads each).
Host pre-transposes x -> xT per batch, slices w_qkv columns / w_proj rows per
head-group.  Each core computes its 6 heads end-to-end and a partial
projection output [S, D]; the host sums the two head-group partials per batch
and adds b_proj.

Device layouts (per core):
  xT   [768, 2048]   (d on partitions)  -> 6 sbuf tiles [128, S]
  qkT  [768(qk cols), S]: rows 0-383 = qT (6 heads x 64), 384-767 = kT.
       6 tiles [128, S]; tile hp (0-2) = qT of head pair hp, tile 3+hp = kT.
  v    natural [S, 6, 65]: per s-tile [128, 6, 65]; col 64 of each head block
       is 1.0 -> the attn @ [v|1] matmul emits the softmax denominator row.
  scores computed TRANSPOSED: sT[kpos, qpos] = k . q  (lhsT=kT, rhs=qT,
       row-tiled pair: head0 at partitions 0-63, head1 at 64-127 run
       concurrently in the PE array).  Softmax denom = row 64 of yT psum.
  yT   [128 (pair y-dims), S] per pair -> proj lhsT directly.
"""

import numpy as np
from contextlib import ExitStack

import concourse.bass as bass
import concourse.bacc as bacc
import concourse.mybir as mybir
from concourse.tile import TileContext

F32 = mybir.dt.float32
F32R = mybir.dt.float32r
BF16 = mybir.dt.bfloat16

D = 768
NCORES = 8
SCALE = 0.125  # 1/sqrt(64)


def build_program(S=2048, use_f32r=True):
    NS = S // 512   # q strips
    NT = S // 128   # s tiles
    DT = D // 128   # d tiles (contraction)

    nc = bacc.Bacc()

    MDT = F32R if use_f32r else F32  # matmul input dtype

    xT = nc.dram_tensor("xT_s", [D, S], MDT, kind="ExternalInput")
    wqkv = nc.dram_tensor("wqkv_s", [D, 1152], MDT, kind="ExternalInput")
    bqk = nc.dram_tensor("bqk_s", [128, 6], F32, kind="ExternalInput")
    bv = nc.dram_tensor("bv_s", [1, 384], MDT, kind="ExternalInput")
    wproj = nc.dram_tensor("wproj_s", [384, D], MDT, kind="ExternalInput")
    out = nc.dram_tensor("out_s", [S, D], F32, kind="ExternalOutput")

    def r(ap):
        return ap

    with TileContext(nc) as tc, ExitStack() as ctx:
        persist = ctx.enter_context(tc.tile_pool(name="persist", bufs=1))

        qkT = [persist.tile([128, S], BF16, tag=f"qkT{i}", name=f"qkT{i}")
               for i in range(6)]
        v_sb = [persist.tile([128, 6, 65], MDT, tag=f"v{i}", name=f"v{i}")
                for i in range(NT)]
        yT = [persist.tile([128, S], MDT, tag=f"yT{i}", name=f"yT{i}")
              for i in range(3)]
        wp = [persist.tile([128, D], MDT, tag=f"wp{i}", name=f"wp{i}")
              for i in range(3)]
        bqk_sb = persist.tile([128, 6], F32, tag="bqk", name="bqk_sb")
        bv_sb = persist.tile([1, 384], MDT, tag="bv", name="bv_sb")
        ones = persist.tile([1, 128], MDT, tag="ones", name="ones_sb")
        ones_f = persist.tile([1, 64], F32, tag="ones_f", name="ones_f_sb")

        maskb = persist.tile([128, 1024], F32, tag="maskb", name="maskb_sb")
        nc.vector.memset(ones[:].bitcast(F32), 1.0)
        nc.vector.memset(ones_f[:], 1.0)
        nc.vector.memset(maskb[:], 0.0)
        # maskb[p, u] = 0 if u >= p + 512 else -30.  The slice
        # maskb[:, 512-128*d : 1024-128*d] is the additive causal mask for a
        # diagonal block at offset d: 0 where q >= k, -30 where masked
        # (exp -> ~1e-13).
        nc.gpsimd.affine_select(
            out=maskb[:], in_=maskb[:],
            compare_op=mybir.AluOpType.is_ge, fill=-30.0, base=-512,
            pattern=[[1, 1024]], channel_multiplier=-1)
        nc.sync.dma_start(out=bqk_sb[:], in_=bqk[:])
        nc.sync.dma_start(out=bv_sb[:], in_=bv[:])
        for i in range(3):
            nc.sync.dma_start(out=wp[i][:], in_=wproj[128 * i:128 * (i + 1), :])
        for st in range(NT):
            nc.vector.memset(v_sb[st][:, :, 64:65].bitcast(F32), 1.0)

        with tc.tile_pool(name="xw", bufs=1) as xw_pool, \
             tc.tile_pool(name="ps1", bufs=3, space="PSUM") as ps1:

            def pe_touch(ap):
                # Tiny self-matmul that makes the PE wait on this tile's
                # producer once, so later real matmuls carry at most ONE sync
                # wait each (fp32r self-loading matmul has 1 LW wait slot).
                t = ps1.tile([1, 1], F32, tag="mm", name="touch")
                nc.tensor.matmul(t[:], ap.bitcast(F32), ap.bitcast(F32),
                                 start=True, stop=True)

            xT_sb = [xw_pool.tile([128, S], MDT, tag=f"xT{i}", name=f"xTs{i}")
                     for i in range(DT)]
            w_sb = [xw_pool.tile([128, 1152], MDT, tag=f"w{i}", name=f"ws{i}")
                    for i in range(DT)]
            for i in range(DT):
                nc.sync.dma_start(out=xT_sb[i][:],
                                  in_=xT[128 * i:128 * (i + 1), :])
                nc.sync.dma_start(out=w_sb[i][:],
                                  in_=wqkv[128 * i:128 * (i + 1), :])
                pe_touch(xT_sb[i][:, 0:1])
                pe_touch(w_sb[i][:, 0:1])
            for i in range(3):
                pe_touch(wp[i][:, 0:1])

            # ---- Phase 1: qkT[c, s] = sum_d wqkv[d, c] * xT[d, s] + bias ----
            for ns in range(NS):
                for ct in range(6):
                    ps = ps1.tile([128, 512], F32, tag="mm", name="ps_qk")
                    for dt_i in range(DT):
                        nc.tensor.matmul(
                            ps[:],
                            r(w_sb[dt_i][:, 128 * ct:128 * ct + 128]),
                            r(xT_sb[dt_i][:, 512 * ns:512 * ns + 512]),
                            start=(dt_i == 0), stop=(dt_i == DT - 1))
                    nc.vector.tensor_scalar_add(
                        qkT[ct][:, 512 * ns:512 * ns + 512], ps[:],
                        bqk_sb[:, ct:ct + 1])

            # ---- Phase 2: v[s, c] = sum_d xT[d, s] * wv[d, c] + bv ----
            for st in range(NT):
                ps = ps1.tile([128, 384], F32, tag="mm", name="ps_v")
                for dt_i in range(DT):
                    nc.tensor.matmul(
                        ps[:],
                        r(xT_sb[dt_i][:, 128 * st:128 * st + 128]),
                        r(w_sb[dt_i][:, 768:1152]),
                        start=(dt_i == 0), stop=False)
                nc.tensor.matmul(ps[:], r(ones[:, 0:128]), r(bv_sb[:]),
                                 start=False, stop=True)
                nc.vector.tensor_copy(
                    v_sb[st][:, :, 0:64],
                    ps[:].rearrange("p (h e) -> p h e", h=6))
                pe_touch(v_sb[st][:, 0, 0:1])

        # ---- Phase 3: attention, scores transposed, per head pair ----
        # k-blocks processed in chunks of 2 (one exp instruction covers a
        # [128, 2, 512] 2-bank PSUM span); the chunk loop is software-
        # pipelined one deep so the PE's scores matmuls for chunk c+1 run
        # while ACT exps chunk c.
        with tc.tile_pool(name="ps_s", bufs=3, space="PSUM") as ps_s, \
             tc.tile_pool(name="ps_y", bufs=2, space="PSUM") as ps_y, \
             tc.tile_pool(name="expp", bufs=6) as expp, \
             tc.tile_pool(name="smp", bufs=3) as smp, \
             tc.tile_pool(name="rcp", bufs=4) as rcp:
            for ns in range(NS):
                q0 = 512 * ns
                for hp in range(3):
                    qt = qkT[hp]
                    kt = qkT[3 + hp]
                    nk = 4 * (ns + 1)
                    nchunk = nk // 2
                    yh = [ps_y.tile([65, 512], F32, tag="yh", name="yh0"),
                          ps_y.tile([65, 512], F32, tag="yh", name="yh1")]

                    def emit_yT(c, ex_pair):
                        for h in range(2):
                            for u in range(2):
                                kb = 2 * c + u
                                c0 = max(0, 128 * kb - q0)
                                nc.tensor.matmul(
                                    yh[h][:, c0:512],
                                    r(v_sb[kb][:, 2 * hp + h, :]),
                                    r(ex_pair[h][:, u, c0:512]),
                                    start=(kb == 0), stop=(kb == nk - 1),
                                    skip_group_check=True)

                    prev = None
                    for c in range(nchunk):
                        diag_c = c >= 2 * ns
                        ex_pair = []
                        for h in range(2):
                            p0 = 64 * h
                            sc2 = ps_s.tile([128, 2, 512], F32, tag="sc",
                                            name="sc2")
                            for u in range(2):
                                kb = 2 * c + u
                                nc.tensor.matmul(
                                    sc2[:, u, :],
                                    r(kt[p0:p0 + 64,
                                         128 * kb:128 * kb + 128]),
                                    r(qt[p0:p0 + 64, q0:q0 + 512]),
                                    start=True, stop=True)
                            ex2 = expp.tile([128, 2, 512], MDT, tag="exp",
                                            name="ex2")
                            if diag_c:
                                sm = smp.tile([128, 2, 512], F32, tag="sm",
                                              name="sm")
                                for u in range(2):
                                    d = 2 * c + u - 4 * ns
                                    nc.vector.scalar_tensor_tensor(
                                        sm[:, u, :], sc2[:, u, :], SCALE,
                                        maskb[:, 512 - 128 * d:
                                              1024 - 128 * d],
                                        op0=mybir.AluOpType.mult,
                                        op1=mybir.AluOpType.add)
                                nc.scalar.activation(
                                    ex2[:, :, :], sm[:, :, :],
                                    mybir.ActivationFunctionType.Exp,
                                    scale=1.0)
                            else:
                                nc.scalar.activation(
                                    ex2[:, :, :], sc2[:, :, :],
                                    mybir.ActivationFunctionType.Exp,
                                    scale=SCALE)
                            ex_pair.append(ex2)
                        if prev is not None:
                            emit_yT(*prev)
                        prev = (c, ex_pair)
                    emit_yT(*prev)
                    for h in range(2):
                        lrow = rcp.tile([1, 512], F32, tag="lrow",
                                        name="lrow")
                        nc.vector.tensor_copy(lrow[:], yh[h][64:65, :])
                        rec = rcp.tile([1, 512], F32, tag="rec", name="rec")
                        nc.vector.reciprocal_approx_fast(rec[:], lrow[:])
                        # broadcast 1/l to 64 partitions on GpSimd (idle
                        # engine; keeps PSUM free and PE unbothered)
                        rb = rcp.tile([128, 512], F32, tag="rb", name="rb",
                                      bufs=2)
                        nc.gpsimd.partition_broadcast(rb[:], rec[:])
                        ys = yT[hp][64 * h:64 * h + 64, q0:q0 + 512]
                        nc.vector.tensor_copy(ys, yh[h][0:64, :])
                        # rb slice picked so both SBUF operands share the
                        # same base partition (ISA requirement)
                        nc.vector.tensor_mul(ys, ys,
                                             rb[64 * h:64 * h + 64, :])

        # ---- Phase 4: partial proj out[s, e] = sum_y yT[y, s] wproj[y, e] --
        with tc.tile_pool(name="ps_o", bufs=2, space="PSUM") as ps_o, \
             tc.tile_pool(name="outp", bufs=2) as outp:
            for st in range(NT):
                pa = ps_o.tile([128, 512], F32, tag="pa", name="pa")
                pb = ps_o.tile([128, 256], F32, tag="pb", name="pb")
                for yt in range(3):
                    nc.tensor.matmul(
                        pa[:], r(yT[yt][:, 128 * st:128 * st + 128]),
                        r(wp[yt][:, 0:512]),
                        start=(yt == 0), stop=(yt == 2))
                for yt in range(3):
                    nc.tensor.matmul(
                        pb[:], r(yT[yt][:, 128 * st:128 * st + 128]),
                        r(wp[yt][:, 512:768]),
                        start=(yt == 0), stop=(yt == 2))
                ot = outp.tile([128, D], F32, tag="ot", name="ot")
                nc.vector.tensor_copy(ot[:, 0:512], pa[:])
                nc.vector.tensor_copy(ot[:, 512:768], pb[:])
                nc.sync.dma_start(out=out[128 * st:128 * st + 128, :],
                                  in_=ot[:])

    nc.finalize()
    return nc


def round_fp32r(a):
    """Round fp32 to fp32r (11 explicit mantissa bits; low 12 bits zero),
    round-to-nearest-even, matching the PE's fp32r input format."""
    a = np.ascontiguousarray(a, dtype=np.float32)
    u = a.view(np.uint32).astype(np.uint64)
    bias = ((u >> 12) & 1) + 0x7FF
    u = ((u + bias) & 0xFFFFF000).astype(np.uint32)
    return u.view(np.float32)


def shard_inputs(x, w_qkv, b_qkv, w_proj):
    """Host-side sharding: returns list of per-core input dicts."""
    in_maps = []
    for core in range(NCORES):
        b, hg = (core // 2) % x.shape[0], core % 2
        cs = slice(384 * hg, 384 * hg + 384)
        xT_s = np.ascontiguousarray(x[b].T).astype(np.float32)
        wqkv_s = np.ascontiguousarray(np.concatenate(
            [w_qkv[:, 0:768][:, cs], w_qkv[:, 768:1536][:, cs],
             w_qkv[:, 1536:2304][:, cs]], axis=1))
        bqk = np.concatenate([b_qkv[0:768][cs], b_qkv[768:1536][cs]])
        bqk_s = np.ascontiguousarray(bqk.reshape(6, 128).T)
        bv_s = np.ascontiguousarray(b_qkv[1536:2304][cs].reshape(1, 384))
        wproj_s = np.ascontiguousarray(w_proj[384 * hg:384 * hg + 384, :])
        in_maps.append({
            "xT_s": round_fp32r(xT_s),
            "wqkv_s": round_fp32r(wqkv_s),
            "bqk_s": bqk_s.astype(np.float32),
            "bv_s": round_fp32r(bv_s),
            "wproj_s": round_fp32r(wproj_s),
        })
    return in_maps


_CACHED = {}


def _get_program():
    if "nc" not in _CACHED:
        _CACHED["nc"] = build_program()
    return _CACHED["nc"]


def kernel(x, w_qkv, b_qkv, w_proj, b_proj):
    from concourse.bass_utils import run_bass_kernel_spmd

    x = np.asarray(x, dtype=np.float32)
    w_qkv = np.asarray(w_qkv, dtype=np.float32)
    b_qkv = np.asarray(b_qkv, dtype=np.float32)
    w_proj = np.asarray(w_proj, dtype=np.float32)
    b_proj = np.asarray(b_proj, dtype=np.float32)

    B, S, dim = x.shape
    nc = _get_program()
    in_maps = shard_inputs(x, w_qkv, b_qkv, w_proj)
    res = run_bass_kernel_spmd(nc, in_maps, core_ids=list(range(NCORES)))
    parts = [m["out_s"] for m in res.results]
    outp = np.empty((B, S, dim), dtype=np.float32)
    for b in range(B):
        outp[b] = parts[2 * b] + parts[2 * b + 1] + b_proj[None, :]
    return outp
